# revision 1
# baseline (speedup 1.0000x reference)
"""GCN (2-layer, Citeseer-style) on 8 Trainium2 NeuronCores.

Strategy (dst-node graph partitioning, per the sharding hint):
  - 50000 nodes -> 392 bins of <=128 nodes (degree-balanced), 49 bins/core.
  - Phase 1: support = x_shard @ W1 per core. x is host-transposed and
    streamed in 512-node strips (bf16 operands, fp32 PSUM accumulation
    over K=3703).
  - The support table is AllGathered in two row-halves (A: first 3200
    rows/core, B: last 3072) so AG_A overlaps the second half of phase 1;
    the A/B split also keeps gather indices within int16 range.
  - Phase 3 (L1 aggregation): per 128-dst tile, two bulk dma_gathers (A/B
    tables) fetch source rows (f32r); a selection matrix
    Mt[e,d] = w_e * (dst_e == d) is built by one fused tensor_scalar
    against a constant iota; Mt.T @ G accumulates in PSUM; + b1,
    leaky_relu -> h1 (SBUF-resident).
  - Phase 5: support2 = h1 @ W2 via PE-transposed h1 tiles; AllGather
    (A/B) of support2 rows padded to 64 floats (256B gather minimum).
  - Phase 6 (L2 aggregation): same Mt machinery at width 6; + b2; batched
    log_softmax; one strided DMA writes the output.
"""
import sys

sys.path.insert(0, "/opt/trn_rl_repo")

import numpy as np

from concourse import bass, bacc, mybir, tile
from concourse.bass_utils import run_bass_kernel_spmd
from concourse.masks import make_identity

F32 = mybir.dt.float32
F32R = mybir.dt.float32r
BF16 = mybir.dt.bfloat16
I16 = mybir.dt.int16

N_NODES = 50000
N_EDGES = 400000
F_IN = 3703
F_HID = 256
F_OUT = 6

CORES = 8
P = 128
TILES = 49                 # dst tiles per core
NPC = TILES * P            # 6272 padded nodes per core
NTOT = CORES * NPC         # 50176
NPCA = 3200                # first-half rows per core (A table)
NPCB = NPC - NPCA          # 3072 (B table)
HALF_A = CORES * NPCA      # 25600 rows in table A (< 32768 for int16)
HALF_B = CORES * NPCB      # 24576 rows in table B
KT = 29                    # ceil(3703/128)
KPAD = KT * P              # 3712
NB = 512                   # phase-1 node block
S2W = 64                   # support2 row padded to 64 floats (256B minimum)
X_BF16 = True              # phase-1 matmul operands in bf16

LAST_RESULT = None         # BassKernelResults of the most recent run
_CACHE = {}                # (CL, CU) -> compiled Bacc


def _idx_cols(CL, CU):
    return TILES * (CL + CU) * 8


def _build(CL, CU, num_devices=CORES, with_ag=True,
           phases=("p1", "p3", "p5", "p6")):
    CH = CL + CU
    idx_cols = _idx_cols(CL, CU)
    XDT = BF16 if X_BF16 else F32

    nc = bacc.Bacc("TRN2", target_bir_lowering=False, debug=False,
                   num_devices=num_devices)

    xT = nc.dram_tensor("xT", [KPAD, NPC], XDT, kind="ExternalInput")
    W1p = nc.dram_tensor("W1p", [KPAD, F_HID], XDT, kind="ExternalInput")
    W2p = nc.dram_tensor("W2p", [F_HID, F_OUT], F32, kind="ExternalInput")
    b1b = nc.dram_tensor("b1b", [P, F_HID], F32, kind="ExternalInput")
    b2b = nc.dram_tensor("b2b", [P, F_OUT], F32, kind="ExternalInput")
    idxd = nc.dram_tensor("idxd", [P, idx_cols], I16, kind="ExternalInput")
    dstd = nc.dram_tensor("dstd", [P, TILES * CH], F32, kind="ExternalInput")
    wd = nc.dram_tensor("wd", [P, TILES * CH], F32, kind="ExternalInput")
    outd = nc.dram_tensor("out", [NPC, F_OUT], F32, kind="ExternalOutput")

    ag1_in = nc.dram_tensor("ag1_in", [NPC, F_HID], F32R, kind="Internal")
    ag1_outA = nc.dram_tensor("ag1_outA", [HALF_A, F_HID], F32R,
                              kind="Internal", addr_space="Shared")
    ag1_outB = nc.dram_tensor("ag1_outB", [HALF_B, F_HID], F32R,
                              kind="Internal", addr_space="Shared")
    ag2_in = nc.dram_tensor("ag2_in", [NPC, S2W], F32R, kind="Internal")
    ag2_outA = nc.dram_tensor("ag2_outA", [HALF_A, S2W], F32R,
                              kind="Internal", addr_space="Shared")
    ag2_outB = nc.dram_tensor("ag2_outB", [HALF_B, S2W], F32R,
                              kind="Internal", addr_space="Shared")

    rg = [list(range(num_devices))]

    def ag(in_ap, out_ap):
        if with_ag:
            nc.gpsimd.collective_compute(
                "AllGather", mybir.AluOpType.bypass, replica_groups=rg,
                ins=[in_ap], outs=[out_ap])
        else:
            nc.sync.dma_start(out=out_ap[0:in_ap.shape[0], :], in_=in_ap)

    with tile.TileContext(nc) as tc:
        with (
            tc.tile_pool(name="res", bufs=1) as rp,
            tc.tile_pool(name="mt", bufs=32) as mp,
        ):
            # ---------- resident constants ----------
            iota_i = rp.tile([P, P], mybir.dt.int32)
            nc.gpsimd.iota(iota_i[:], pattern=[[1, P]], base=0,
                           channel_multiplier=0)
            iota_f = rp.tile([P, P], F32)
            nc.vector.tensor_copy(iota_f[:], iota_i[:])
            ident = rp.tile([P, P], F32)
            make_identity(nc, ident[:])

            w2sb = rp.tile([P, 2, F_OUT], F32R)
            nc.gpsimd.dma_start(
                out=w2sb[:], in_=W2p[:, :].rearrange("(k p) n -> p k n", p=P))
            b1sb = rp.tile([P, F_HID], F32)
            nc.sync.dma_start(out=b1sb[:], in_=b1b[:, :])
            b2sb = rp.tile([P, F_OUT], F32)
            nc.sync.dma_start(out=b2sb[:], in_=b2b[:, :])
            idxsb = rp.tile([P, idx_cols], I16)
            nc.sync.dma_start(out=idxsb[:], in_=idxd[:, :])
            dstsb = rp.tile([P, TILES * CH], F32)
            nc.sync.dma_start(out=dstsb[:], in_=dstd[:, :])
            wsb = rp.tile([P, TILES * CH], F32)
            nc.sync.dma_start(out=wsb[:], in_=wd[:, :])

            # ---------- phase 1: support = x @ W1 ----------
            if "p1" in phases:
                with (
                    tc.tile_pool(name="p1w", bufs=1) as p1w,
                    tc.tile_pool(name="p1x", bufs=3) as p1x,
                    tc.tile_pool(name="p1ps", bufs=4, space="PSUM") as p1ps,
                ):
                    w1sb = p1w.tile([P, KT, F_HID], XDT)
                    nc.sync.dma_start(
                        out=w1sb[:],
                        in_=W1p[:, :].rearrange("(k p) n -> p k n", p=P))
                    blocks = []
                    b0 = 0
                    while b0 < NPC:
                        bsz = min(NB, (NPCA - b0) if b0 < NPCA else (NPC - b0))
                        blocks.append((b0, bsz))
                        b0 += bsz
                    for b0, bsz in blocks:
                        xsb = p1x.tile([P, KT, bsz], XDT, tag="xsb")
                        nc.sync.dma_start(
                            out=xsb[:],
                            in_=xT[:, b0:b0 + bsz].rearrange(
                                "(k p) n -> p k n", p=P))
                        nm = bsz // P
                        sup = p1x.tile([P, nm, F_HID], F32R, tag="sup")
                        for m in range(nm):
                            ps = p1ps.tile([P, F_HID], F32, tag="p1")
                            for k in range(KT):
                                nc.tensor.matmul(
                                    ps[:],
                                    lhsT=xsb[:, k, m * P:(m + 1) * P],
                                    rhs=w1sb[:, k, :],
                                    start=(k == 0), stop=(k == KT - 1))
                            nc.vector.tensor_copy(sup[:, m, :], ps[:])
                        nc.sync.dma_start(
                            out=ag1_in[b0:b0 + bsz, :].rearrange(
                                "(m p) f -> p m f", p=P),
                            in_=sup[:])
                        if b0 + bsz == NPCA:
                            ag(ag1_in[0:NPCA, :], ag1_outA[:, :])
                    ag(ag1_in[NPCA:NPC, :], ag1_outB[:, :])
            elif "p3" in phases:
                ag(ag1_in[0:NPCA, :], ag1_outA[:, :])
                ag(ag1_in[NPCA:NPC, :], ag1_outB[:, :])

            with (
                tc.tile_pool(name="big", bufs=1) as bigp,
                tc.tile_pool(name="work", bufs=3) as wp,
                tc.tile_pool(name="ps", bufs=2, space="PSUM") as pp,
            ):
                # ---------- phase 3: L1 aggregation ----------
                h1 = bigp.tile([P, TILES * F_HID], F32)
                if "p3" not in phases:
                    nc.gpsimd.memset(h1[:], 0.0)

                def agg_tiles(tname, tabA, tabB, esz, psum_w, out_cb):
                    CHW = CL + CU
                    NCHL = TILES * CL
                    NCHU = TILES * CU
                    LBASE = 0
                    UBASE = NCHL * 8
                    gbL, gbU = {}, {}
                    nxt = [0, 0]

                    def issue(stream, k):
                        nch = NCHL if stream == 0 else NCHU
                        base = LBASE if stream == 0 else UBASE
                        tab = tabA if stream == 0 else tabB
                        c0 = k * 8
                        n = min(8, nch - c0)
                        gb = wp.tile([P, n, esz], F32R,
                                     tag="%s%d" % (tname, stream))
                        nc.gpsimd.dma_gather(
                            out_ap=gb[:], in_ap=tab,
                            idxs_ap=idxsb[:, base + c0 * 8:base + (c0 + n) * 8],
                            num_idxs=n * P, num_idxs_reg=n * P,
                            elem_size=esz)
                        (gbL if stream == 0 else gbU)[k] = gb

                    for t in range(TILES):
                        while nxt[0] <= ((t + 1) * CL - 1) // 8:
                            issue(0, nxt[0])
                            nxt[0] += 1
                        while nxt[1] <= ((t + 1) * CU - 1) // 8:
                            issue(1, nxt[1])
                            nxt[1] += 1
                        ps = pp.tile([P, psum_w], F32, tag="pa%d" % psum_w)
                        for c in range(CHW):
                            col = t * CHW + c
                            if c < CL:
                                g = t * CL + c
                                gb, slot = gbL[g // 8], g % 8
                            else:
                                g = t * CU + (c - CL)
                                gb, slot = gbU[g // 8], g % 8
                            mt = mp.tile([P, P], F32R, tag="mt")
                            nc.vector.tensor_scalar(
                                out=mt[:], in0=iota_f[:],
                                scalar1=dstsb[:, col:col + 1],
                                scalar2=wsb[:, col:col + 1],
                                op0=mybir.AluOpType.is_equal,
                                op1=mybir.AluOpType.mult)
                            nc.tensor.matmul(
                                ps[:], lhsT=mt[:], rhs=gb[:, slot, 0:psum_w],
                                start=(c == 0), stop=(c == CHW - 1))
                        out_cb(t, ps)

                def l1_out(t, ps):
                    hz = wp.tile([P, F_HID], F32, tag="hz")
                    nc.vector.tensor_tensor(
                        out=hz[:], in0=ps[:], in1=b1sb[:],
                        op=mybir.AluOpType.add)
                    nc.scalar.activation(
                        h1[:, t * F_HID:(t + 1) * F_HID], hz[:],
                        mybir.ActivationFunctionType.Lrelu, alpha=0.01)

                if "p3" in phases:
                    agg_tiles("gb1", ag1_outA[:, :], ag1_outB[:, :],
                              F_HID, F_HID, l1_out)

                # ---------- phase 5: support2 = h1 @ W2 ----------
                for t in (range(TILES) if "p5" in phases else []):
                    ps2 = pp.tile([P, F_OUT], F32, tag="p5")
                    for kk in range(2):
                        tp = pp.tile([P, P], F32, tag="tp")
                        nc.tensor.transpose(
                            out=tp[:],
                            in_=h1[:, t * F_HID + kk * P:
                                   t * F_HID + (kk + 1) * P],
                            identity=ident[:])
                        h1T = wp.tile([P, P], F32R, tag="h1T")
                        nc.vector.tensor_copy(h1T[:], tp[:])
                        nc.tensor.matmul(ps2[:], lhsT=h1T[:],
                                         rhs=w2sb[:, kk, :],
                                         start=(kk == 0), stop=(kk == 1))
                    s2 = wp.tile([P, F_OUT], F32R, tag="s2")
                    nc.vector.tensor_copy(s2[:], ps2[:])
                    nc.sync.dma_start(out=ag2_in[t * P:(t + 1) * P, 0:F_OUT],
                                      in_=s2[:])
                if "p5" in phases and "p6" in phases:
                    ag(ag2_in[0:NPCA, :], ag2_outA[:, :])
                    ag(ag2_in[NPCA:NPC, :], ag2_outB[:, :])

                # ---------- phase 6: L2 aggregation ----------
                zall = bigp.tile([P, TILES * F_OUT], F32)

                def l2_out(t, ps):
                    nc.vector.tensor_tensor(
                        out=zall[:, t * F_OUT:(t + 1) * F_OUT], in0=ps[:],
                        in1=b2sb[:], op=mybir.AluOpType.add)

                if "p6" in phases:
                    agg_tiles("gb2", ag2_outA[:, :], ag2_outB[:, :],
                              S2W, F_OUT, l2_out)
                else:
                    nc.gpsimd.memset(zall[:], 0.0)

                # ---------- phase 7: batched log_softmax + output ----------
                zv = zall[:].rearrange("p (t f) -> p t f", f=F_OUT)
                mx = wp.tile([P, TILES], F32, tag="mx")
                nc.vector.tensor_reduce(out=mx[:], in_=zv,
                                        op=mybir.AluOpType.max,
                                        axis=mybir.AxisListType.X)
                tsub = bigp.tile([P, TILES, F_OUT], F32)
                nc.vector.tensor_tensor(
                    out=tsub[:], in0=zv,
                    in1=mx[:][:, :, None].to_broadcast([P, TILES, F_OUT]),
                    op=mybir.AluOpType.subtract)
                ex = wp.tile([P, TILES, F_OUT], F32, tag="ex")
                nc.scalar.activation(ex[:], tsub[:],
                                     mybir.ActivationFunctionType.Exp)
                sm = wp.tile([P, TILES], F32, tag="sm")
                nc.vector.tensor_reduce(out=sm[:], in_=ex[:],
                                        op=mybir.AluOpType.add,
                                        axis=mybir.AxisListType.X)
                ls = wp.tile([P, TILES], F32, tag="ls")
                nc.scalar.activation(ls[:], sm[:],
                                     mybir.ActivationFunctionType.Ln)
                res = bigp.tile([P, TILES, F_OUT], F32)
                nc.vector.tensor_tensor(
                    out=res[:], in0=tsub[:],
                    in1=ls[:][:, :, None].to_broadcast([P, TILES, F_OUT]),
                    op=mybir.AluOpType.subtract)
                nc.sync.dma_start(
                    out=outd[:, :].rearrange("(t p) f -> p t f", p=P),
                    in_=res[:])

    nc.compile()
    return nc


def _preprocess(x, edge_src, edge_dst, edge_weight, W1, b1, W2, b2):
    x = np.asarray(x, dtype=np.float32)
    edge_src = np.asarray(edge_src, dtype=np.int64)
    edge_dst = np.asarray(edge_dst, dtype=np.int64)
    edge_weight = np.asarray(edge_weight, dtype=np.float32)
    W1 = np.asarray(W1, dtype=np.float32)
    b1 = np.asarray(b1, dtype=np.float32)
    W2 = np.asarray(W2, dtype=np.float32)
    b2 = np.asarray(b2, dtype=np.float32)

    NBINS = CORES * TILES
    deg = np.bincount(edge_dst, minlength=N_NODES)

    # degree-balanced assignment of nodes to bins (LPT with 128-node cap)
    import heapq
    order = np.argsort(-deg, kind="stable")
    heap = [(0, b) for b in range(NBINS)]
    heapq.heapify(heap)
    counts = np.zeros(NBINS, dtype=np.int64)
    node_row = np.empty(N_NODES, dtype=np.int64)
    for nid in order:
        while True:
            load, b = heapq.heappop(heap)
            if counts[b] < P:
                break
        core, t = b // TILES, b % TILES
        node_row[nid] = core * NPC + t * P + counts[b]
        counts[b] += 1
        if counts[b] < P:
            heapq.heappush(heap, (load + int(deg[nid]), b))

    src_row = node_row[edge_src]
    dst_row = node_row[edge_dst]
    core_e = dst_row // NPC
    t_e = (dst_row % NPC) // P
    lane_d = dst_row % P
    src_core = src_row // NPC
    src_local = src_row % NPC
    half_e = (src_local >= NPCA).astype(np.int64)
    loc_src = np.where(half_e == 0, src_core * NPCA + src_local,
                       src_core * NPCB + (src_local - NPCA))

    # position of each edge within its (core,tile,half) run
    key = (core_e * TILES + t_e) * 2 + half_e
    sort_i = np.argsort(key, kind="stable")
    ks = key[sort_i]
    cnt = np.bincount(ks, minlength=NBINS * 2)
    starts = np.zeros(NBINS * 2, dtype=np.int64)
    starts[1:] = np.cumsum(cnt)[:-1]
    pos_sorted = np.arange(N_EDGES) - starts[ks]
    pos = np.empty(N_EDGES, dtype=np.int64)
    pos[sort_i] = pos_sorted

    nL = cnt[0::2].reshape(CORES, TILES)
    nU = cnt[1::2].reshape(CORES, TILES)
    CL = max(1, int(np.ceil(nL.max() / P)))
    CU = max(1, int(np.ceil(nU.max() / P)))
    CH = CL + CU
    idx_cols = _idx_cols(CL, CU)

    g_stream = np.where(half_e == 0, t_e * CL + pos // P,
                        t_e * CU + pos // P)
    sbase = np.where(half_e == 0, 0, TILES * CL * 8)
    lane_s = pos % P
    idx_col = sbase + g_stream * 8 + lane_s // 16
    idx_par = lane_s % 16

    idx_arr = np.zeros((CORES, 16, idx_cols), dtype=np.int16)
    idx_arr[core_e, idx_par, idx_col] = loc_src.astype(np.int16)

    # per-(tile,chunk) Mt data
    c_e = np.where(half_e == 0, pos // P, CL + pos // P)
    lane_e = pos % P
    dcol = t_e * CH + c_e
    dst_arr = np.zeros((CORES, P, TILES * CH), dtype=np.float32)
    w_arr = np.zeros((CORES, P, TILES * CH), dtype=np.float32)
    dst_arr[core_e, lane_e, dcol] = lane_d.astype(np.float32)
    w_arr[core_e, lane_e, dcol] = edge_weight

    if X_BF16:
        import ml_dtypes
        xdt = ml_dtypes.bfloat16
    else:
        xdt = np.float32

    W1p = np.zeros((KPAD, F_HID), dtype=np.float32)
    W1p[:F_IN] = W1
    W1p_c = np.ascontiguousarray(W1p.astype(xdt))
    b1b = np.broadcast_to(b1, (P, F_HID)).copy()
    b2b = np.broadcast_to(b2, (P, F_OUT)).copy()

    in_maps = []
    row_node = np.full(NTOT, -1, dtype=np.int64)
    row_node[node_row] = np.arange(N_NODES)
    for c in range(CORES):
        rows = row_node[c * NPC:(c + 1) * NPC]
        xc = np.zeros((NPC, F_IN), dtype=np.float32)
        occ = rows >= 0
        xc[occ] = x[rows[occ]]
        xTc = np.zeros((KPAD, NPC), dtype=xdt)
        xTc[:F_IN] = xc.T.astype(xdt) if X_BF16 else xc.T
        in_maps.append(dict(
            xT=xTc,
            W1p=W1p_c,
            W2p=W2,
            b1b=b1b,
            b2b=b2b,
            idxd=np.tile(idx_arr[c], (8, 1)),
            dstd=dst_arr[c],
            wd=w_arr[c],
        ))
    return in_maps, node_row, CL, CU


def kernel(**inputs):
    global LAST_RESULT
    in_maps, node_row, CL, CU = _preprocess(**inputs)
    key = (CL, CU)
    if key not in _CACHE:
        _CACHE[key] = _build(CL, CU)
    nc = _CACHE[key]
    res = run_bass_kernel_spmd(nc, in_maps, core_ids=list(range(CORES)))
    LAST_RESULT = res
    allout = np.concatenate([res.results[c]["out"] for c in range(CORES)],
                            axis=0)
    return np.ascontiguousarray(allout[node_row]).astype(np.float32)

